# revision 35
# baseline (speedup 1.0000x reference)
"""Multi-head attention (B=2, S=2048, D=2048, H=16, DH=128, RoPE, non-causal)
on 8 Trainium2 NeuronCores.

Sharding: 2-way data parallel on batch x 4-way tensor parallel on heads.
Core c handles batch c//4 and heads (c%4)*4 .. (c%4)*4+4.

Compute dtype: fp16 on the TensorEngine (fp32 PSUM accumulation), which runs
at the same PE rate as bf16 with 8 more mantissa bits, and qualifies DVE
tensor ops for the 2x 16-bit fast path.

Per-core kernel:
  1. xT resident in SBUF (loaded once, slice-major so sc=0 lands first).
  2. QK projections, head-dim-major QT/KT[dh, s], fused RoPE via sign-folded
     sin table and partition-strided rotate-half SBUF DMA (on the otherwise
     idle GpSimd DMA queue).
  3. V projection, seq-major V[s, dh*4], from the resident xT (no re-DMA).
  4. Attention, transpose-free: scoresT[k, q] on PE, one exp ACTIVATE per
     [128,1024] PSUM tile (fp16 out), softmax denominators via wide fp16
     DVE accumulation (2x 16-bit DVE mode) + all-ones matmul,
     reciprocal_approx_fast, and the normalize multiply reads the attnV
     PSUM directly (no evacuation).
  5. Per-head 8-way AllToAll (~12us vs ~41us for the AllGather equivalent):
     each core sends peer-q-slice blocks duplicated into both batch-groups'
     shard slots, reads back its group's 4 blocks via a cc_rank-based
     dynamic row offset. The last head's A2A is split into two half-width
     collectives so the wo waves pipeline against them.
  6. Output projection: 16 half-chunks [128 q, 512 d] on 4 PSUM pipelines,
     each a single 16-matmul accumulation chain over all 16 heads, ordered
     h-major so ~3/4 of each wave's matmuls run during the final A2A's
     flight. ACT-copied to SBUF fp16 and DMA'd out.
Host gathers the 8 disjoint [512, 2048] fp16 output shards and upcasts.
"""
import numpy as np

B, S, D, H = 2, 2048, 2048, 16
DH = 128
HLOC = 4                 # heads per core
NCORES = 8
GROUPS = [[0, 1, 2, 3], [4, 5, 6, 7]]
SCALE = 1.0 / np.sqrt(DH)
KT = D // 128            # 16 contraction tiles over the model dim
SC = S // 512            # 4 chunks of 512 along seq
ST = S // 128            # 16 seq tiles of 128
QROWS = S // 4           # 512 output rows per core

_BUILT = None


def _build():
    import concourse.bass as bass
    import concourse.tile as tile
    from concourse import bacc, bass_isa, mybir

    F32 = mybir.dt.float32
    F16 = mybir.dt.float16
    EXPF = mybir.ActivationFunctionType.Exp
    COPYF = mybir.ActivationFunctionType.Copy

    nc = bacc.Bacc("TRN2", target_bir_lowering=False, debug=False,
                   num_devices=NCORES)

    xT_d = nc.dram_tensor("xT", [D, S], F16, kind="ExternalInput").ap()
    wqk_d = nc.dram_tensor("wqkT", [D, 2 * HLOC * DH], F16,
                           kind="ExternalInput").ap()
    wv_d = nc.dram_tensor("wvT", [D, HLOC * DH], F16,
                          kind="ExternalInput").ap()
    woT_d = nc.dram_tensor("woT", [D, D], F16, kind="ExternalInput").ap()
    cosT_d = nc.dram_tensor("cosT", [DH, S], F16, kind="ExternalInput").ap()
    sinT_d = nc.dram_tensor("sinTs", [DH, S], F16, kind="ExternalInput").ap()
    ones_d = nc.dram_tensor("ones", [128, 128], F16,
                            kind="ExternalInput").ap()
    out_d = nc.dram_tensor("out", [QROWS, D], F16, kind="ExternalOutput").ap()

    with tile.TileContext(nc) as tc:
        with (
            tc.tile_pool(name="dram", bufs=1, space="DRAM") as dram,
            tc.tile_pool(name="onesp", bufs=1) as onesp,
        ):
            ones_sb = onesp.tile([128, 128], F16)

            a2a_ins = []
            a2a_outs = []
            for h in range(HLOC):
                a2a_ins.append(dram.tile([8 * DH, QROWS], F16,
                                         name=f"a2ain{h}"))
                a2a_outs.append(dram.tile([8 * DH, QROWS], F16,
                                          name=f"a2aout{h}"))
            a2a3_in = [dram.tile([8 * DH, QROWS // 2], F16,
                                 name=f"a2a3in{half}") for half in range(2)]
            a2a3_out = [dram.tile([8 * DH, QROWS // 2], F16,
                                  name=f"a2a3out{half}") for half in range(2)]
            A2AGRP = [[0, 1, 2, 3, 4, 5, 6, 7]]

            warm_in = dram.tile([128, 128], F16, name="warmin")
            warm_out = dram.tile([128, 128], F16, name="warmout")

            def _warmup_collective():
                """Tiny dummy AllToAll absorbing the ~50us first-collective
                runtime cost; issued after the hot-path lead-in DMAs."""
                nc.sync.dma_start(warm_in[:], ones_d[:])
                nc.gpsimd.collective_compute(
                    "AllToAll", mybir.AluOpType.bypass,
                    replica_groups=A2AGRP,
                    ins=[warm_in.opt()], outs=[warm_out.opt()],
                )

            with (
                tc.tile_pool(name="qkr", bufs=1) as qkrp,
                tc.tile_pool(name="vper", bufs=1) as vperp,
            ):
                # persistent RoPE'd Q/K, head-dim-major: [dh=128, s=2048]
                QTr = [qkrp.tile([DH, S], F16, name=f"qtr{h}")
                       for h in range(HLOC)]
                KTr = [qkrp.tile([DH, S], F16, name=f"ktr{h}")
                       for h in range(HLOC)]
                # persistent V, seq-major: 16 tiles [128 s, 512 = 4 heads]
                V_sb = [vperp.tile([128, HLOC * DH], F16, name=f"v{st}")
                        for st in range(ST)]

                # ============ Phase A: QKV projections + RoPE ============
                with (
                    tc.tile_pool(name="xsb", bufs=1) as xsbp,
                    tc.tile_pool(name="wqk", bufs=1) as wqkp,
                    tc.tile_pool(name="wv", bufs=1) as wvp,
                    tc.tile_pool(name="cs", bufs=1) as csp,
                    tc.tile_pool(name="ropew", bufs=2) as ropep,
                    tc.tile_pool(name="psA", bufs=2, space="PSUM") as psA,
                ):
                    x_sb = [xsbp.tile([128, S], F16, name=f"x{kt}")
                            for kt in range(KT)]
                    wqk_sb = [wqkp.tile([128, 2 * HLOC * DH], F16,
                                        name=f"wqk{kt}") for kt in range(KT)]
                    wv_sb = [wvp.tile([128, HLOC * DH], F16,
                                      name=f"wv{kt}") for kt in range(KT)]
                    cos_sb = csp.tile([DH, S], F16)
                    sin_sb = csp.tile([DH, S], F16)

                    # DMA order = first-needed-first: wqk[0], then x slices
                    # sc-major so the sc=0 QK matmuls start ~2MB in.
                    nc.sync.dma_start(wqk_sb[0][:, 0:128],
                                      wqk_d[0:128, 0:128])
                    first_x = True
                    for sc in range(SC):
                        ssl = slice(sc * 512, (sc + 1) * 512)
                        for kt in range(KT):
                            nc.sync.dma_start(
                                x_sb[kt][:, ssl],
                                xT_d[kt * 128:(kt + 1) * 128, ssl])
                            if first_x:
                                nc.sync.dma_start(wqk_sb[0][:, 128:],
                                                  wqk_d[0:128, 128:])
                                first_x = False
                            if sc == 0 and kt + 1 < KT:
                                nc.sync.dma_start(
                                    wqk_sb[kt + 1][:],
                                    wqk_d[(kt + 1) * 128:(kt + 2) * 128, :])
                        if sc == 0:
                            nc.sync.dma_start(cos_sb[:], cosT_d[:])
                            nc.sync.dma_start(sin_sb[:], sinT_d[:])
                            nc.sync.dma_start(ones_sb[:], ones_d[:])
                            _warmup_collective()
                        if sc == 1:
                            for kt in range(KT):
                                nc.sync.dma_start(
                                    wv_sb[kt][:],
                                    wv_d[kt * 128:(kt + 1) * 128, :])

                    # ---- QK projections + RoPE evacuation ----
                    for sc in range(SC):
                        ssl = slice(sc * 512, (sc + 1) * 512)
                        pss = [psA.tile([128, 512], F32, tag="qkps",
                                        bufs=8, name=f"qkps{sc}_{t}")
                               for t in range(2 * HLOC)]
                        for kt in range(KT):
                            for t in range(2 * HLOC):
                                nc.tensor.matmul(
                                    pss[t][:],
                                    wqk_sb[kt][:, t * 128:(t + 1) * 128],
                                    x_sb[kt][:, ssl],
                                    start=(kt == 0), stop=(kt == KT - 1))
                        # RoPE: dst = psum*cos + rot(psum)*sin_signfolded
                        for t in range(2 * HLOC):
                            h, isk = t // 2, t % 2
                            dst = (KTr[h] if isk else QTr[h])
                            plain = ropep.tile([128, 512], F16, tag="plain",
                                               bufs=6, name=f"pl{sc}_{t}")
                            nc.scalar.copy(plain[:], pss[t][:])
                            tmpc = ropep.tile([128, 512], F16, tag="tmpc",
                                              bufs=6, name=f"tc{sc}_{t}")
                            nc.vector.tensor_mul(tmpc[:], pss[t][:],
                                                 cos_sb[:, ssl])
                            rot = ropep.tile([128, 512], F16, tag="rot",
                                             bufs=6, name=f"ro{sc}_{t}")
                            nc.gpsimd.dma_start(rot[0:64, :],
                                                plain[1::2, :])
                            nc.gpsimd.dma_start(rot[64:128, :],
                                                plain[0::2, :])
                            rot2 = ropep.tile([128, 512], F16, tag="rot2",
                                              bufs=6, name=f"ro2{sc}_{t}")
                            nc.vector.tensor_mul(rot2[:], rot[:],
                                                 sin_sb[:, ssl])
                            nc.vector.tensor_add(dst[:, ssl], tmpc[:],
                                                 rot2[:])

                    # ---- V projection from resident x. i-outer: each
                    # 16-matmul chain completes early so its evacuation
                    # overlaps the next chain instead of bunching at the
                    # phase end (where the PSUM pool boundary would stall
                    # the first attention matmul behind the last copy).
                    for half in range(2):
                        for i in range(8):
                            pvs = psA.tile([128, 512], F32, tag="qkps",
                                           bufs=8, name=f"vps{half}_{i}")
                            scol = half * 1024 + i * 128
                            for kt in range(KT):
                                nc.tensor.matmul(
                                    pvs[:],
                                    x_sb[kt][:, scol:scol + 128],
                                    wv_sb[kt][:],
                                    start=(kt == 0), stop=(kt == KT - 1))
                            nc.vector.tensor_copy(V_sb[half * 8 + i][:],
                                                  pvs[:])

                # ============ Phase B: attention + A2A + wo ============
                with (
                    tc.tile_pool(name="wo", bufs=1) as wop,
                    tc.tile_pool(name="ao", bufs=1) as aop,
                    tc.tile_pool(name="outTp", bufs=1) as outTp,
                    tc.tile_pool(name="atw", bufs=3) as atw,
                    tc.tile_pool(name="sacc", bufs=2) as saccp,
                    tc.tile_pool(name="psB", bufs=1, space="PSUM") as psB,
                ):
                    # attention outputs, head-dim-major
                    outT_sb = [outTp.tile([DH, S], F16, name=f"ot{h}")
                               for h in range(HLOC)]
                    # woT resident fp16, loaded during attention
                    wo_sb = [wop.tile([128, D], F16, name=f"wos{g}")
                             for g in range(16)]
                    for g in range(16):
                        nc.sync.dma_start(
                            wo_sb[g][:], woT_d[g * 128:(g + 1) * 128, :])

                    ao_sb = [[aop.tile([128, QROWS], F16, name=f"ao{h}_{j}")
                              for j in range(4)] for h in range(HLOC)]

                    # which 4-core batch group am I in (0 or 1): row base of
                    # my group's blocks in the 8-way A2A output, in f32 rows
                    grp = nc.sync.cc_rank(
                        replica_groups=[[0, 4], [1, 5], [2, 6], [3, 7]])
                    arow = grp * 512

                    def _tail(h, kp, scp, outp, sacc, s, is_last):
                        """exp + attnV accumulation + wide softmax-sum."""
                        ex = atw.tile([128, 1024], F16, tag=f"ex{s}",
                                      name=f"ex{h}_{kp}_{s}")
                        nc.scalar.activation(ex[:], scp[:], EXPF, scale=SCALE)
                        for j in range(2):
                            st = 2 * kp + j
                            nc.tensor.matmul(
                                outp[:], V_sb[st][:, h * 128:(h + 1) * 128],
                                ex[:, j * 512:(j + 1) * 512],
                                start=(kp == 0 and j == 0),
                                stop=(is_last and j == 1))
                        if kp == 0:
                            nc.vector.tensor_copy(sacc[:], ex[:])
                        else:
                            nc.vector.tensor_add(sacc[:], sacc[:], ex[:])

                    def _flush(h, qsl, outp, sacc, s):
                        """denominator + normalize, reading outp PSUM
                        directly: the unnormalized sums overflow fp16, so
                        they must be scaled in f32 before any 16-bit cast."""
                        fold = atw.tile([128, 512], F16, tag=f"fold{s}",
                                        bufs=2, name=f"fo{h}_{qsl.start}")
                        nc.vector.tensor_add(fold[:], sacc[:, 0:512],
                                             sacc[:, 512:1024])
                        sump = psB.tile([128, 512], F32, tag="small",
                                        bufs=2, name=f"sm{h}_{qsl.start}")
                        nc.tensor.matmul(sump[:], ones_sb[:], fold[:],
                                         start=True, stop=True)
                        rec = atw.tile([128, 512], F32, tag=f"rec{s}",
                                       bufs=2, name=f"rc{h}_{qsl.start}")
                        nc.vector.reciprocal_approx_fast(rec[:], sump[:])
                        nc.vector.tensor_mul(outT_sb[h][:, qsl],
                                             outp[:], rec[:])

                    for h in range(HLOC):
                        for qp in range(SC // 2):
                            qcs = (2 * qp, 2 * qp + 1)
                            qsls = [slice(qc * 512, (qc + 1) * 512)
                                    for qc in qcs]
                            outps = [psB.tile([128, 512], F32, tag=f"out{s}",
                                              bufs=1, name=f"aO{h}_{qcs[s]}")
                                     for s in range(2)]
                            saccs = [saccp.tile([128, 1024], F16,
                                                tag=f"sacc{s}", bufs=2,
                                                name=f"sA{h}_{qcs[s]}")
                                     for s in range(2)]
                            prevs = [None, None]
                            for kp in range(ST // 2):
                                scps = []
                                for s in range(2):
                                    scp = psB.tile(
                                        [128, 1024], F32, tag=f"sc{s}",
                                        bufs=1, name=f"sc{h}_{qcs[s]}_{kp}")
                                    for j in range(2):
                                        k0 = (2 * kp + j) * 128
                                        nc.tensor.matmul(
                                            scp[:, j * 512:(j + 1) * 512],
                                            KTr[h][:, k0:k0 + 128],
                                            QTr[h][:, qsls[s]],
                                            start=True, stop=True)
                                    scps.append(scp)
                                for s in range(2):
                                    if prevs[s] is not None:
                                        _tail(h, prevs[s][0], prevs[s][1],
                                              outps[s], saccs[s], s, False)
                                    prevs[s] = (kp, scps[s])
                            for s in range(2):
                                _tail(h, prevs[s][0], prevs[s][1],
                                      outps[s], saccs[s], s, True)
                                _flush(h, qsls[s], outps[s], saccs[s], s)
                        # head done: assemble + exchange q-slices (all at
                        # head end: assembly DMAs concurrent with an active
                        # collective slow it ~2.5x), fetch my group's 4
                        # blocks back via rank-dependent row offset.
                        # The last head's A2A is split into two half-width
                        # collectives so the wo waves pipeline against them.
                        if h < HLOC - 1:
                            for p in range(8):
                                nc.sync.dma_start(
                                    a2a_ins[h][p * 128:(p + 1) * 128, :],
                                    outT_sb[h][:, (p % 4) * 512:
                                                ((p % 4) + 1) * 512])
                            nc.gpsimd.collective_compute(
                                "AllToAll", mybir.AluOpType.bypass,
                                replica_groups=A2AGRP,
                                ins=[a2a_ins[h].opt()],
                                outs=[a2a_outs[h].opt()],
                            )
                            ao32 = a2a_outs[h].bitcast(F32)
                            for j in range(4):
                                nc.sync.dma_start(
                                    ao_sb[h][j][:].bitcast(F32),
                                    ao32[bass.ds(arow + j * 128, 128), :])
                        else:
                            for half in range(2):
                                for p in range(8):
                                    c0 = (p % 4) * 512 + half * 256
                                    nc.sync.dma_start(
                                        a2a3_in[half][p * 128:(p + 1) * 128,
                                                      :],
                                        outT_sb[h][:, c0:c0 + 256])
                            for half in range(2):
                                nc.gpsimd.collective_compute(
                                    "AllToAll", mybir.AluOpType.bypass,
                                    replica_groups=A2AGRP,
                                    ins=[a2a3_in[half].opt()],
                                    outs=[a2a3_out[half].opt()],
                                )
                                ao32 = a2a3_out[half].bitcast(F32)
                                for j in range(4):
                                    nc.sync.dma_start(
                                        ao_sb[h][j][:, half * 256:
                                                    half * 256 + 256]
                                        .bitcast(F32),
                                        ao32[bass.ds(arow + j * 128, 128),
                                             :])

                    # ---- wo: 16 half-chunks [128 q, 512 d] on 4 PSUM
                    # pipelines (8 banks), 16-matmul chains ordered h-major
                    # so each wave's h=0..2 matmuls (96 of 128) can run
                    # during the final A2A's flight time.
                    for wave in range(2):
                        pA = psB.tile([128, 1024], F32, tag="sc0",
                                      name=f"wpsA{wave}")
                        pB = psB.tile([128, 1024], F32, tag="sc1",
                                      name=f"wpsB{wave}")
                        pC0 = psB.tile([128, 512], F32, tag="out0",
                                       name=f"wpsC0{wave}")
                        pC1 = psB.tile([128, 512], F32, tag="out1",
                                       name=f"wpsC1{wave}")
                        pD0 = psB.tile([128, 512], F32, tag="small",
                                       bufs=2, name=f"wpsD0{wave}")
                        pD1 = psB.tile([128, 512], F32, tag="small",
                                       bufs=2, name=f"wpsD1{wave}")
                        aps = [pA[:, 0:512], pA[:, 512:1024],
                               pB[:, 0:512], pB[:, 512:1024],
                               pC0[:], pC1[:], pD0[:], pD1[:]]
                        specs = []
                        for hc in range(8):
                            g = wave * 8 + hc
                            specs.append((aps[hc], g // 4, g % 4))
                        for h in range(HLOC):
                            for ap, qt, dcol in specs:
                                for j in range(4):
                                    nc.tensor.matmul(
                                        ap,
                                        ao_sb[h][j][:,
                                                    qt * 128:(qt + 1) * 128],
                                        wo_sb[4 * j + h][:, dcol * 512:
                                                         (dcol + 1) * 512],
                                        start=(h == 0 and j == 0),
                                        stop=(h == 3 and j == 3))
                        for hc, (ap, qt, dcol) in enumerate(specs):
                            och = atw.tile([128, 512], F16, tag="oc",
                                           bufs=4, name=f"oc{wave}_{hc}")
                            nc.scalar.activation(och[:], ap, COPYF)
                            nc.sync.dma_start(
                                out_d[qt * 128:(qt + 1) * 128,
                                      dcol * 512:(dcol + 1) * 512],
                                och[:])

    nc.compile()
    return nc


def _prep_inputs(x, wq, wk, wv, wo, cos, sin):
    """Host-side sharding/layout prep. Returns per-core input dicts."""
    f16 = np.float16

    woT = np.ascontiguousarray(wo.T).astype(f16)          # [E, D]
    cosT = np.ascontiguousarray(cos[:S, :DH].T).astype(f16)  # [DH, S]
    sinT = np.ascontiguousarray(sin[:S, :DH].T)
    sinTs = sinT.copy()
    sinTs[:DH // 2] *= -1.0                               # sign-folded rotate
    sinTs = sinTs.astype(f16)
    ones = np.ones((128, 128), f16)
    xTs = [np.ascontiguousarray(x[b].T).astype(f16) for b in range(B)]

    in_maps = []
    for c in range(NCORES):
        hsel = slice((c % 4) * HLOC * DH, ((c % 4) + 1) * HLOC * DH)
        wq_c = wq[hsel, :]                                # [512, D]
        wk_c = wk[hsel, :]
        qk_cols = np.empty((2 * HLOC * DH, D), np.float32)
        for h in range(HLOC):
            qk_cols[(2 * h) * DH:(2 * h + 1) * DH] = \
                wq_c[h * DH:(h + 1) * DH]
            qk_cols[(2 * h + 1) * DH:(2 * h + 2) * DH] = \
                wk_c[h * DH:(h + 1) * DH]
        wqkT = np.ascontiguousarray(qk_cols.T).astype(f16)    # [D, 1024]
        wvT = np.ascontiguousarray(wv[hsel, :].T).astype(f16)  # [D, 512]
        in_maps.append({
            "xT": xTs[c // 4],
            "wqkT": wqkT,
            "wvT": wvT,
            "woT": woT,
            "cosT": cosT,
            "sinTs": sinTs,
            "ones": ones,
        })
    return in_maps


def kernel(x, wq, wk, wv, wo, cos, sin):
    global _BUILT
    from concourse.bass_utils import run_bass_kernel_spmd

    if _BUILT is None:
        _BUILT = _build()
    nc = _BUILT

    in_maps = _prep_inputs(
        np.asarray(x, np.float32), np.asarray(wq, np.float32),
        np.asarray(wk, np.float32), np.asarray(wv, np.float32),
        np.asarray(wo, np.float32), np.asarray(cos, np.float32),
        np.asarray(sin, np.float32))

    res = run_bass_kernel_spmd(nc, in_maps, core_ids=list(range(NCORES)))

    out = np.empty((B, S, D), np.float32)
    for c in range(NCORES):
        out[c // 4, (c % 4) * QROWS:((c % 4) + 1) * QROWS, :] = \
            np.asarray(res.results[c]["out"], np.float32)
    return out
